# revision 21
# baseline (speedup 1.0000x reference)
"""DynamicTriModalFusion Trainium2 kernel (8-core data-parallel over batch).

Math (per token t, modalities i,j in {flow,wave,wind}):
    sims[i,j]  = cos(S_i, S_j) / T
    w[i,:]     = softmax_j(sims[i,:])
    fused_i    = sum_j w[i,j] * S_j
    p_i        = fused_i @ W_i^T + b_i
    out_i      = LayerNorm(S_i + p_i) * g_i + beta_i
Returns (out [3,B,T,D], weights [3,B,T,3]).

v2 design (measured-cost driven):
- groups of 128 tokens processed in PAIRS (shared instructions halve per-op
  dispatch overhead on the small softmax chain).
- cosine dots: diagonal via ACT Square+accum, cross via DVE
  scalar_tensor_tensor (fused multiply+row-reduce).
- 1/sqrt computed as exp(-0.5*ln(.)) so ACT stays in one function table
  (exp/ln/square/identity/copy) - table reloads measured at 1.3us each.
- modality combine fused with the transpose on the PE: fusedT accumulates
  matmul(lhsT=S_j chunk, rhs=[diag(w_0j)|diag(w_1j)|diag(w_2j)]), N=384 so
  fp32r streams at 1 cycle/row. All 18 diag matrices built in ONE DVE
  tensor_tensor with stride-0 broadcast APs.
- PSUM->SBUF evacuation on the DMA engines (per chunk, overlapping PE).
- LayerNorm: residual add as one paired TT, stats via bn_stats, normalize via
  ACT Identity(x*rstd + (-mu*rstd)).
- all matmuls fp32r (e8m11): host pre-rounds inputs; on-chip producers emit
  fp32r. PSUM accumulate stays fp32.
- proj_b / ln_g / ln_b applied only when not 0/1/0 (checked on host).
"""
import os
import sys

sys.path.insert(0, "/opt/trn_rl_repo")
import numpy as np

import concourse.bass as bass
import concourse.tile as tile
from concourse import mybir
from concourse.masks import make_identity
from concourse.vector_clock import ScopedClock

F32 = mybir.dt.float32
F32R = mybir.dt.float32r
AF = mybir.ActivationFunctionType
OP = mybir.AluOpType

TEMP_SCALE = 2.0  # 1/TEMPERATURE
EPS_COS = 1e-8
EPS_LN = 1e-5

# ---------------------------------------------------------------------------
# walrus in this container allows only ONE sem wait per instruction; Tile
# attaches several. Patch the tail drain and add a post-pass that moves extra
# waits onto single-wait NoOps just before the instruction on the same engine.


def _patched_drain_and_barrier(self, tick_clock, wait_clock):
    probe = self.nc.sync.nop(nofuse=True)
    wait_clock.add_sem_waits(probe.ins, ScopedClock({None: tick_clock.global_clock}))
    waits = list(probe.ins.sync_info.on_wait or []) if probe.ins.sync_info else []
    if len(waits) > 1:
        probe.ins.sync_info.on_wait = waits[:1]
        for w in waits[1:]:
            extra = self.nc.sync.nop(nofuse=True)
            if extra.ins.sync_info is None:
                extra.ins.sync_info = mybir.SyncInfo(on_wait=[], on_update=[])
            extra.ins.sync_info.on_wait = [w]
    self.nc.sync.drain()
    self.nc.all_engine_barrier()
    assert self.sems is not None
    popped = self.nc._tile_sem_poison_stack.pop()
    assert popped is self._sem_poison
    self.nc.clear_and_free_semaphores(list(self.sems.allocated().values()))
    self.nc.all_engine_barrier()


tile.TileContext._drain_and_barrier = _patched_drain_and_barrier

_nopcnt = [0]


def legalize_waits(nc, max_waits=1):
    n_split = 0
    for f in nc.m.functions:
        for b in f.blocks:
            dirty = False
            newl = []
            for inst in b.instructions:
                si = inst.sync_info
                waits = list(si.on_wait) if (si and si.on_wait) else []
                if len(waits) > max_waits:
                    for w in waits[: len(waits) - max_waits]:
                        _nopcnt[0] += 1
                        nop = mybir.InstNoOp(name=f"I-waitnop-{_nopcnt[0]}")
                        nop.engine = inst.engine
                        nop.sync_info = mybir.SyncInfo(on_wait=[w], on_update=[])
                        newl.append(nop)
                    si.on_wait = waits[len(waits) - max_waits :]
                    dirty = True
                    n_split += 1
                newl.append(inst)
            if dirty:
                b.instructions = newl
    return n_split


# ---------------------------------------------------------------------------


def round_fp32r(x):
    """Round fp32 to e8m11 (fp32 with low 12 mantissa bits zero, RNE)."""
    u = np.ascontiguousarray(x, dtype=np.float32).view(np.uint32)
    lsb = (u >> 12) & 1
    r = (u + 0x7FF + lsb) & 0xFFFFF000
    return r.astype(np.uint32).view(np.float32)


def _view(t, off_slots, dims):
    """Raw strided AP view over a [P, nslots] tile: dims = [(step, count)] in
    elements."""
    return bass.AP(
        tensor=t.tensor,
        offset=t.offset + off_slots * 4,
        ap=[t.ap[0]] + [list(d) for d in dims],
    )


def _boost(r, by=100000):
    try:
        r.ins.bass_priority = (r.ins.bass_priority or 0) - by
    except Exception:
        pass
    return r


def build_nc(B_loc, T, D, use_pb, use_g, use_b, reps=1):
    P = 128
    NC4 = D // P
    groups_per_b = T // P
    npairs = B_loc * T // (2 * P)
    nc = bass.Bass(trn_type="TRN2")

    s3 = nc.dram_tensor("s3", [3, B_loc, T, D], F32R, kind="ExternalInput")
    wT = nc.dram_tensor("wT", [3, D, D], F32R, kind="ExternalInput")
    pb = nc.dram_tensor("pb", [3, D], F32R, kind="ExternalInput") if use_pb else None
    lg = nc.dram_tensor("lg", [3, D], F32, kind="ExternalInput") if use_g else None
    lb = nc.dram_tensor("lb", [3, D], F32, kind="ExternalInput") if use_b else None

    out = nc.dram_tensor("out", [3, B_loc, T, D], F32, kind="ExternalOutput")
    wout = nc.dram_tensor("wout", [3, B_loc, T, 3], F32, kind="ExternalOutput")

    with tile.TileContext(nc) as tc:
        with (
            tc.tile_pool(name="const", bufs=1) as const_pool,
            tc.tile_pool(name="sS", bufs=4) as s_pool,
            tc.tile_pool(name="sm", bufs=4) as small_pool,
            tc.tile_pool(name="dg", bufs=3) as diag_pool,
            tc.tile_pool(name="fT", bufs=6) as fT_pool,
            tc.tile_pool(name="xx", bufs=6) as x_pool,
            tc.tile_pool(name="oo", bufs=6) as o_pool,
            tc.tile_pool(name="scr", bufs=3) as scr_pool,
            tc.tile_pool(name="psF", bufs=5, space="PSUM") as psF_pool,
            tc.tile_pool(name="psP", bufs=3, space="PSUM") as psP_pool,
        ):
            # ---- constants
            wT_sb = const_pool.tile([P, 3, NC4, D], F32R, tag="wT")
            nc.sync.dma_start(
                out=wT_sb, in_=wT[:].rearrange("i (c p) e -> p i c e", p=P)
            )
            ident = const_pool.tile([P, P], F32, tag="ident")
            make_identity(nc, ident)
            eps_ln = const_pool.tile([P, 1], F32, tag="epsln")
            nc.vector.memset(eps_ln, EPS_LN)
            if use_pb:
                ones_row = const_pool.tile([1, P], F32R, tag="ones")
                nc.vector.memset(ones_row, 1.0)
                pb_sb = const_pool.tile([1, 3, D], F32R, tag="pbt")
                nc.sync.dma_start(out=pb_sb, in_=pb[:][None, :, :])
            if use_g:
                lg_sb = const_pool.tile([P, 3, D], F32, tag="lg")
                nc.sync.dma_start(out=lg_sb, in_=lg[:].partition_broadcast(P))
            if use_b:
                lb_sb = const_pool.tile([P, 3, D], F32, tag="lb")
                nc.sync.dma_start(out=lb_sb, in_=lb[:].partition_broadcast(P))

            for _rep in range(reps):
              for pr in range(npairs):
                b, gg = divmod(2 * pr, groups_per_b)
                t0 = gg * P

                # ---- load both groups' S: [tok, 2g, 3j, D], fp32r
                st = s_pool.tile([P, 2, 3, D], F32R, tag="s")
                for g in range(2):
                    nc.sync.dma_start(
                        out=st[:, g, :, :],
                        in_=s3[:, b, t0 + g * P : t0 + (g + 1) * P, :].rearrange(
                            "j p d -> p j d"
                        ),
                    )
                stf = st.bitcast(F32)

                # slot tile (f32), per-group stride 12 so the diagonal
                # slots {0,4,8} extract via rearrange(b=4):
                # dots 0:24 | qq 24:48 | e 48:72 | esum 72:78 | w 80:104
                # | bn 104:152 [k*8] | ws 152:200 [k*8]  (k = 2*i+g)
                sm = small_pool.tile([P, 200], F32, tag="sm")
                scrA = scr_pool.tile([P, D], F32, tag="scrA")
                scrD = scr_pool.tile([P, D], F32, tag="scrD")

                def gview(base, n=9):
                    return sm[:, base : base + 24].rearrange(
                        "p (g k) -> p g k", k=12
                    )[:, :, 0:n]

                dots = gview(0)
                nn = sm[:, 0:24].rearrange("p (g a b) -> p g a b", g=2, b=4)[
                    :, :, :, 0
                ]

                # ---- dots
                for g in range(2):
                    for i in range(3):
                        _boost(nc.scalar.activation(
                            out=scrA,
                            in_=stf[:, g, i, :],
                            func=AF.Square,
                            accum_out=sm[:, 12 * g + 4 * i : 12 * g + 4 * i + 1],
                        ))
                    for i, j in [(0, 1), (0, 2), (1, 2)]:
                        _boost(nc.vector.scalar_tensor_tensor(
                            out=scrD,
                            in0=stf[:, g, i, :],
                            scalar=1.0,
                            in1=stf[:, g, j, :],
                            op0=OP.mult,
                            op1=OP.mult,
                            accum_out=sm[
                                :, 12 * g + 3 * i + j : 12 * g + 3 * i + j + 1
                            ],
                        ))
                # mirror cross slots (j,i) <- (i,j), both groups per op
                for dst, srcs in [(3, 1), (6, 2), (7, 5)]:
                    _boost(nc.gpsimd.tensor_copy(
                        out=gview(0)[:, :, dst : dst + 1],
                        in_=gview(0)[:, :, srcs : srcs + 1],
                    ))

                # ---- rden = (max(ss_ii*ss_jj, EPS^2))^(-1/2) via ln/exp
                qq9 = gview(24)
                _boost(nc.vector.tensor_tensor(
                    out=qq9.rearrange("p g (i j) -> p g i j", i=3),
                    in0=nn[:, :, :, None].broadcast_to([P, 2, 3, 3]),
                    in1=nn[:, :, None, :].broadcast_to([P, 2, 3, 3]),
                    op=OP.mult,
                ))
                _boost(nc.vector.tensor_scalar_max(qq9, qq9, float(EPS_COS) ** 2))
                _boost(nc.scalar.activation(out=qq9, in_=qq9, func=AF.Ln))
                _boost(nc.scalar.activation(out=qq9, in_=qq9, func=AF.Exp, scale=-0.5))

                # ---- softmax: e = exp(2*dots*rden); w = e * (1/sum_j e)
                e9 = gview(48)
                _boost(nc.vector.tensor_tensor(out=e9, in0=dots, in1=qq9, op=OP.mult))
                _boost(nc.scalar.activation(out=e9, in_=e9, func=AF.Exp, scale=TEMP_SCALE))
                esum = sm[:, 72:78].rearrange("p (g i) -> p g i", g=2)
                _boost(nc.vector.tensor_reduce(
                    out=esum,
                    in_=e9.rearrange("p g (i j) -> p g i j", i=3),
                    op=OP.add,
                    axis=mybir.AxisListType.X,
                ))
                _boost(nc.vector.reciprocal(out=esum, in_=esum))
                w9 = gview(80)
                _boost(nc.vector.tensor_tensor(
                    out=w9.rearrange("p g (i j) -> p g i j", i=3),
                    in0=e9.rearrange("p g (i j) -> p g i j", i=3),
                    in1=esum[:, :, :, None].broadcast_to([P, 2, 3, 3]),
                    op=OP.mult,
                ))
                # weights out: w [p, i, j] -> wout[i, b, t, j]
                for g in range(2):
                    nc.sync.dma_start(
                        out=wout[:, b, t0 + g * P : t0 + (g + 1) * P, :].rearrange(
                            "i p j -> p i j"
                        ),
                        in_=sm[:, 80 + 12 * g : 80 + 12 * g + 9].rearrange(
                            "p (i j) -> p i j", i=3
                        ),
                    )

                # ---- diag matrices: diags[p, g, j, i, q] = ident[p,q]*w[p,g,3i+j]
                diags = diag_pool.tile([P, 2, 3, 3, P], F32R, tag="diags")
                for g in range(2):
                    _boost(nc.vector.tensor_tensor(
                        out=diags[:, g],
                        in0=ident[:, None, None, :].broadcast_to([P, 3, 3, P]),
                        in1=sm[:, 80 + 12 * g : 80 + 12 * g + 9]
                        .rearrange("p (i j) -> p j i", i=3)[:, :, :, None]
                        .broadcast_to([P, 3, 3, P]),
                        op=OP.mult,
                    ))

                # ---- PE combine + chunked DMA evac
                fT_sbs = []
                for g in range(2):
                    fT_sb = fT_pool.tile([P, NC4, 3, P], F32R, tag="fT")
                    for c in range(NC4):
                        # one PSUM bank per chunk so evac frees banks early
                        fT_ps = psF_pool.tile([P, 512], F32, tag="psF")
                        for j in range(3):
                            nc.tensor.matmul(
                                fT_ps[:, 0 : 3 * P],
                                lhsT=st[:, g, j, c * P : (c + 1) * P],
                                rhs=diags[:, g, j, :, :],
                                start=(j == 0),
                                stop=(j == 2),
                            )
                        # evac chunk c: [fused_0 | fused_1 | fused_2]^T
                        _boost(nc.scalar.copy(
                            out=fT_sb[:, c, :, :],
                            in_=fT_ps[:, 0 : 3 * P],
                        ))
                    fT_sbs.append(fT_sb)

                # ---- projections + residual + bn stats
                x_tiles = []
                for i in range(3):
                    x_t = x_pool.tile([P, 2, D], F32, tag="x")
                    x_tiles.append(x_t)
                    for g in range(2):
                        p_g = psP_pool.tile([P, D], F32, tag="psP")
                        for c in range(NC4):
                            nc.tensor.matmul(
                                p_g,
                                lhsT=fT_sbs[g][:, c, i, :],
                                rhs=wT_sb[:, i, c, :],
                                start=(c == 0),
                                stop=(c == NC4 - 1) and not use_pb,
                            )
                        if use_pb:
                            nc.tensor.matmul(
                                p_g,
                                lhsT=ones_row,
                                rhs=pb_sb[:, i, :],
                                start=False,
                                stop=True,
                            )
                        nc.vector.tensor_tensor(
                            out=x_t[:, g, :],
                            in0=stf[:, g, i, :],
                            in1=p_g,
                            op=OP.add,
                        )
                        k = 2 * i + g
                        nc.vector.bn_stats(
                            out=sm[:, 104 + k * 8 : 104 + k * 8 + 6],
                            in_=x_t[:, g, :],
                        )

                # ---- LN stats from bn outputs [cnt, m_e, cv_e, cnt, m_o, cv_o]
                # ws blocks of 8: 0: msum->bias, 2: d->d2h, 4: cvsum->var, 6: rstd
                bnv = sm[:, 104:152].rearrange("p (k s) -> p k s", s=8)
                wsv = sm[:, 152:200].rearrange("p (k s) -> p k s", s=8)

                def bn(s):
                    return bnv[:, :, s : s + 1]

                def ws(s):
                    return wsv[:, :, s : s + 1]

                nc.vector.tensor_tensor(out=ws(0), in0=bn(1), in1=bn(4), op=OP.add)
                nc.vector.tensor_tensor(
                    out=ws(2), in0=bn(1), in1=bn(4), op=OP.subtract
                )
                nc.vector.tensor_tensor(out=ws(4), in0=bn(2), in1=bn(5), op=OP.add)
                nc.vector.scalar_tensor_tensor(
                    out=ws(2), in0=ws(2), scalar=0.5, in1=ws(2),
                    op0=OP.mult, op1=OP.mult,
                )
                nc.vector.tensor_scalar_mul(ws(4), ws(4), 1.0 / D)
                nc.vector.scalar_tensor_tensor(
                    out=ws(4), in0=ws(2), scalar=0.5, in1=ws(4),
                    op0=OP.mult, op1=OP.add,
                )
                # rstd = exp(-0.5*ln(var + eps))
                nc.scalar.activation(out=ws(6), in_=ws(4), func=AF.Ln, bias=eps_ln)
                nc.scalar.activation(out=ws(6), in_=ws(6), func=AF.Exp, scale=-0.5)
                # bias = (msum * -0.5) * rstd
                nc.vector.scalar_tensor_tensor(
                    out=ws(0), in0=ws(0), scalar=-0.5, in1=ws(6),
                    op0=OP.mult, op1=OP.mult,
                )

                # ---- normalize + store
                for i in range(3):
                    o_t = o_pool.tile([P, 2, D], F32, tag="o")
                    for g in range(2):
                        k = 2 * i + g
                        nc.scalar.activation(
                            out=o_t[:, g, :],
                            in_=x_tiles[i][:, g, :],
                            func=AF.Identity,
                            scale=sm[:, 152 + k * 8 + 6 : 152 + k * 8 + 7],
                            bias=sm[:, 152 + k * 8 : 152 + k * 8 + 1],
                        )
                    if use_g:
                        nc.vector.tensor_mul(
                            o_t, o_t,
                            lg_sb[:, i, None, :].broadcast_to([P, 2, D]),
                        )
                    if use_b:
                        nc.vector.tensor_add(
                            o_t, o_t,
                            lb_sb[:, i, None, :].broadcast_to([P, 2, D]),
                        )
                    nc.sync.dma_start(
                        out=out[i, b, t0 : t0 + 2 * P, :].rearrange(
                            "(g p) d -> p g d", p=P
                        ),
                        in_=o_t,
                    )

    return nc


_last_info = {}


def _install_ntff_hook():
    """antenv.axon_hooks is absent in this image; recreate the registry so
    run_bass_kernel_spmd(trace=True) can reach the ctypes NTFF profiler."""
    import types

    if "antenv.axon_hooks" in sys.modules:
        return
    try:
        from trn_agent_boot.trn_boot import _ntff_profile_via_ctypes

        hook = _ntff_profile_via_ctypes("/opt/axon/libaxon_pjrt.so")
    except Exception:
        hook = None
    m = types.ModuleType("antenv.axon_hooks")
    _h = [hook]
    m.get_axon_ntff_profile_hook = lambda: _h[0]
    m.set_axon_ntff_profile_hook = lambda h: _h.__setitem__(0, h)
    sys.modules["antenv.axon_hooks"] = m


def kernel(s_flow, s_wave, s_wind, proj_w, proj_b, ln_g, ln_b):
    from concourse.bass_utils import run_bass_kernel_spmd

    B, T, D = s_flow.shape
    NCORES = 8
    B_loc = B // NCORES

    S = np.stack([np.asarray(s_flow), np.asarray(s_wave), np.asarray(s_wind)], axis=0)
    S_r = round_fp32r(S)
    WT = np.ascontiguousarray(np.transpose(np.asarray(proj_w, np.float32), (0, 2, 1)))
    WT_r = round_fp32r(WT)

    pb = np.asarray(proj_b, np.float32)
    lg = np.asarray(ln_g, np.float32)
    lb = np.asarray(ln_b, np.float32)
    use_pb = not np.all(pb == 0.0)
    use_g = not np.all(lg == 1.0)
    use_b = not np.all(lb == 0.0)

    reps = int(os.environ.get("KERNEL_REPS", "1"))
    nc = build_nc(B_loc, T, D, use_pb, use_g, use_b, reps=reps)
    legalize_waits(nc)

    in_maps = []
    for c in range(NCORES):
        m = {
            "s3": np.ascontiguousarray(S_r[:, c * B_loc : (c + 1) * B_loc]),
            "wT": WT_r,
        }
        if use_pb:
            m["pb"] = round_fp32r(pb)
        if use_g:
            m["lg"] = lg
        if use_b:
            m["lb"] = lb
        in_maps.append(m)

    trace = bool(int(os.environ.get("KERNEL_TRACE", "0")))
    if trace:
        _install_ntff_hook()
        from concourse import bass_utils as _bu

        _bu.upload_artifacts = lambda d: "local://skipped"
        tmpdir = "/root/problem/ntff_out"
        os.makedirs(tmpdir, exist_ok=True)
        res = run_bass_kernel_spmd(
            nc, in_maps, list(range(NCORES)), trace=True, tmpdir=tmpdir
        )
    else:
        res = run_bass_kernel_spmd(nc, in_maps, list(range(NCORES)))
    _last_info["exec_time_ns"] = res.exec_time_ns
    _last_info["results"] = res

    out = np.concatenate([r["out"] for r in res.results], axis=1)
    wout = np.concatenate([r["wout"] for r in res.results], axis=1)
    return out, wout


# revision 22
# speedup vs baseline: 1.0997x; 1.0997x over previous
"""DynamicTriModalFusion Trainium2 kernel (8-core data-parallel over batch).

Math (per token t, modalities i,j in {flow,wave,wind}):
    sims[i,j]  = cos(S_i, S_j) / T
    w[i,:]     = softmax_j(sims[i,:])
    fused_i    = sum_j w[i,j] * S_j
    p_i        = fused_i @ W_i^T + b_i
    out_i      = LayerNorm(S_i + p_i) * g_i + beta_i
Returns (out [3,B,T,D], weights [3,B,T,3]).

v2 design (measured-cost driven):
- groups of 128 tokens processed in PAIRS (shared instructions halve per-op
  dispatch overhead on the small softmax chain).
- cosine dots: diagonal via ACT Square+accum, cross via DVE
  scalar_tensor_tensor (fused multiply+row-reduce).
- 1/sqrt computed as exp(-0.5*ln(.)) so ACT stays in one function table
  (exp/ln/square/identity/copy) - table reloads measured at 1.3us each.
- modality combine fused with the transpose on the PE: fusedT accumulates
  matmul(lhsT=S_j chunk, rhs=[diag(w_0j)|diag(w_1j)|diag(w_2j)]), N=384 so
  fp32r streams at 1 cycle/row. All 18 diag matrices built in ONE DVE
  tensor_tensor with stride-0 broadcast APs.
- PSUM->SBUF evacuation on the DMA engines (per chunk, overlapping PE).
- LayerNorm: residual add as one paired TT, stats via bn_stats, normalize via
  ACT Identity(x*rstd + (-mu*rstd)).
- all matmuls fp32r (e8m11): host pre-rounds inputs; on-chip producers emit
  fp32r. PSUM accumulate stays fp32.
- proj_b / ln_g / ln_b applied only when not 0/1/0 (checked on host).
"""
import os
import sys

sys.path.insert(0, "/opt/trn_rl_repo")
import numpy as np

import concourse.bass as bass
import concourse.tile as tile
from concourse import mybir
from concourse.masks import make_identity
from concourse.vector_clock import ScopedClock

F32 = mybir.dt.float32
F32R = mybir.dt.float32r
AF = mybir.ActivationFunctionType
OP = mybir.AluOpType

TEMP_SCALE = 2.0  # 1/TEMPERATURE
EPS_COS = 1e-8
EPS_LN = 1e-5

# ---------------------------------------------------------------------------
# walrus in this container allows only ONE sem wait per instruction; Tile
# attaches several. Patch the tail drain and add a post-pass that moves extra
# waits onto single-wait NoOps just before the instruction on the same engine.


def _patched_drain_and_barrier(self, tick_clock, wait_clock):
    probe = self.nc.sync.nop(nofuse=True)
    wait_clock.add_sem_waits(probe.ins, ScopedClock({None: tick_clock.global_clock}))
    waits = list(probe.ins.sync_info.on_wait or []) if probe.ins.sync_info else []
    if len(waits) > 1:
        probe.ins.sync_info.on_wait = waits[:1]
        for w in waits[1:]:
            extra = self.nc.sync.nop(nofuse=True)
            if extra.ins.sync_info is None:
                extra.ins.sync_info = mybir.SyncInfo(on_wait=[], on_update=[])
            extra.ins.sync_info.on_wait = [w]
    self.nc.sync.drain()
    self.nc.all_engine_barrier()
    assert self.sems is not None
    popped = self.nc._tile_sem_poison_stack.pop()
    assert popped is self._sem_poison
    self.nc.clear_and_free_semaphores(list(self.sems.allocated().values()))
    self.nc.all_engine_barrier()


tile.TileContext._drain_and_barrier = _patched_drain_and_barrier

_nopcnt = [0]


def legalize_waits(nc, max_waits=1):
    n_split = 0
    for f in nc.m.functions:
        for b in f.blocks:
            dirty = False
            newl = []
            for inst in b.instructions:
                si = inst.sync_info
                waits = list(si.on_wait) if (si and si.on_wait) else []
                if len(waits) > max_waits:
                    for w in waits[: len(waits) - max_waits]:
                        _nopcnt[0] += 1
                        nop = mybir.InstNoOp(name=f"I-waitnop-{_nopcnt[0]}")
                        nop.engine = inst.engine
                        nop.sync_info = mybir.SyncInfo(on_wait=[w], on_update=[])
                        newl.append(nop)
                    si.on_wait = waits[len(waits) - max_waits :]
                    dirty = True
                    n_split += 1
                newl.append(inst)
            if dirty:
                b.instructions = newl
    return n_split


# ---------------------------------------------------------------------------


def round_fp32r(x):
    """Round fp32 to e8m11 (fp32 with low 12 mantissa bits zero, RNE)."""
    u = np.ascontiguousarray(x, dtype=np.float32).view(np.uint32)
    lsb = (u >> 12) & 1
    r = (u + 0x7FF + lsb) & 0xFFFFF000
    return r.astype(np.uint32).view(np.float32)


def _view(t, off_slots, dims):
    """Raw strided AP view over a [P, nslots] tile: dims = [(step, count)] in
    elements."""
    return bass.AP(
        tensor=t.tensor,
        offset=t.offset + off_slots * 4,
        ap=[t.ap[0]] + [list(d) for d in dims],
    )


def _boost(r, by=100000):
    try:
        r.ins.bass_priority = (r.ins.bass_priority or 0) - by
    except Exception:
        pass
    return r


def build_nc(B_loc, T, D, use_pb, use_g, use_b, reps=1):
    P = 128
    NC4 = D // P
    groups_per_b = T // P
    npairs = B_loc * T // (2 * P)
    nc = bass.Bass(trn_type="TRN2")

    s3 = nc.dram_tensor("s3", [3, B_loc, T, D], F32R, kind="ExternalInput")
    wT = nc.dram_tensor("wT", [3, D, D], F32R, kind="ExternalInput")
    pb = nc.dram_tensor("pb", [3, D], F32R, kind="ExternalInput") if use_pb else None
    lg = nc.dram_tensor("lg", [3, D], F32, kind="ExternalInput") if use_g else None
    lb = nc.dram_tensor("lb", [3, D], F32, kind="ExternalInput") if use_b else None

    out = nc.dram_tensor("out", [3, B_loc, T, D], F32, kind="ExternalOutput")
    wout = nc.dram_tensor("wout", [3, B_loc, T, 3], F32, kind="ExternalOutput")

    with tile.TileContext(nc) as tc:
        with (
            tc.tile_pool(name="const", bufs=1) as const_pool,
            tc.tile_pool(name="sS", bufs=4) as s_pool,
            tc.tile_pool(name="sm", bufs=4) as small_pool,
            tc.tile_pool(name="dg", bufs=3) as diag_pool,
            tc.tile_pool(name="fT", bufs=6) as fT_pool,
            tc.tile_pool(name="xx", bufs=6) as x_pool,
            tc.tile_pool(name="oo", bufs=6) as o_pool,
            tc.tile_pool(name="scr", bufs=3) as scr_pool,
            tc.tile_pool(name="psF", bufs=5, space="PSUM") as psF_pool,
            tc.tile_pool(name="psP", bufs=3, space="PSUM") as psP_pool,
        ):
            # ---- constants
            wT_sb = const_pool.tile([P, 3, NC4, D], F32R, tag="wT")
            nc.sync.dma_start(
                out=wT_sb, in_=wT[:].rearrange("i (c p) e -> p i c e", p=P)
            )
            ident = const_pool.tile([P, P], F32, tag="ident")
            make_identity(nc, ident)
            eps_ln = const_pool.tile([P, 1], F32, tag="epsln")
            nc.vector.memset(eps_ln, EPS_LN)
            if use_pb:
                ones_row = const_pool.tile([1, P], F32R, tag="ones")
                nc.vector.memset(ones_row, 1.0)
                pb_sb = const_pool.tile([1, 3, D], F32R, tag="pbt")
                nc.sync.dma_start(out=pb_sb, in_=pb[:][None, :, :])
            if use_g:
                lg_sb = const_pool.tile([P, 3, D], F32, tag="lg")
                nc.sync.dma_start(out=lg_sb, in_=lg[:].partition_broadcast(P))
            if use_b:
                lb_sb = const_pool.tile([P, 3, D], F32, tag="lb")
                nc.sync.dma_start(out=lb_sb, in_=lb[:].partition_broadcast(P))

            for _rep in range(reps):
              for pr in range(npairs):
                b, gg = divmod(2 * pr, groups_per_b)
                t0 = gg * P

                # ---- load both groups' S: [tok, 2g, 3j, D], fp32r
                st = s_pool.tile([P, 2, 3, D], F32R, tag="s")
                for g in range(2):
                    _boost(nc.sync.dma_start(
                        out=st[:, g, :, :],
                        in_=s3[:, b, t0 + g * P : t0 + (g + 1) * P, :].rearrange(
                            "j p d -> p j d"
                        ),
                    ), by=200000)
                stf = st.bitcast(F32)

                # slot tile (f32), per-group stride 12 so the diagonal
                # slots {0,4,8} extract via rearrange(b=4):
                # dots 0:24 | qq 24:48 | e 48:72 | esum 72:78 | w 80:104
                # | bn 104:152 [k*8] | ws 152:200 [k*8]  (k = 2*i+g)
                sm = small_pool.tile([P, 200], F32, tag="sm")
                scrA = scr_pool.tile([P, D], F32, tag="scrA")
                scrD = scr_pool.tile([P, D], F32, tag="scrD")

                def gview(base, n=9):
                    return sm[:, base : base + 24].rearrange(
                        "p (g k) -> p g k", k=12
                    )[:, :, 0:n]

                dots = gview(0)
                nn = sm[:, 0:24].rearrange("p (g a b) -> p g a b", g=2, b=4)[
                    :, :, :, 0
                ]

                # ---- dots
                for g in range(2):
                    for i in range(3):
                        _boost(nc.scalar.activation(
                            out=scrA,
                            in_=stf[:, g, i, :],
                            func=AF.Square,
                            accum_out=sm[:, 12 * g + 4 * i : 12 * g + 4 * i + 1],
                        ))
                    for i, j in [(0, 1), (0, 2), (1, 2)]:
                        _boost(nc.vector.scalar_tensor_tensor(
                            out=scrD,
                            in0=stf[:, g, i, :],
                            scalar=1.0,
                            in1=stf[:, g, j, :],
                            op0=OP.mult,
                            op1=OP.mult,
                            accum_out=sm[
                                :, 12 * g + 3 * i + j : 12 * g + 3 * i + j + 1
                            ],
                        ))
                # mirror cross slots (j,i) <- (i,j), both groups per op
                for dst, srcs in [(3, 1), (6, 2), (7, 5)]:
                    _boost(nc.gpsimd.tensor_copy(
                        out=gview(0)[:, :, dst : dst + 1],
                        in_=gview(0)[:, :, srcs : srcs + 1],
                    ))

                # ---- rden = (max(ss_ii*ss_jj, EPS^2))^(-1/2) via ln/exp
                qq9 = gview(24)
                _boost(nc.vector.tensor_tensor(
                    out=qq9.rearrange("p g (i j) -> p g i j", i=3),
                    in0=nn[:, :, :, None].broadcast_to([P, 2, 3, 3]),
                    in1=nn[:, :, None, :].broadcast_to([P, 2, 3, 3]),
                    op=OP.mult,
                ))
                _boost(nc.vector.tensor_scalar_max(qq9, qq9, float(EPS_COS) ** 2))
                _boost(nc.scalar.activation(out=qq9, in_=qq9, func=AF.Ln))
                _boost(nc.scalar.activation(out=qq9, in_=qq9, func=AF.Exp, scale=-0.5))

                # ---- softmax: e = exp(2*dots*rden); w = e * (1/sum_j e)
                e9 = gview(48)
                _boost(nc.vector.tensor_tensor(out=e9, in0=dots, in1=qq9, op=OP.mult))
                _boost(nc.scalar.activation(out=e9, in_=e9, func=AF.Exp, scale=TEMP_SCALE))
                esum = sm[:, 72:78].rearrange("p (g i) -> p g i", g=2)
                _boost(nc.vector.tensor_reduce(
                    out=esum,
                    in_=e9.rearrange("p g (i j) -> p g i j", i=3),
                    op=OP.add,
                    axis=mybir.AxisListType.X,
                ))
                _boost(nc.vector.reciprocal(out=esum, in_=esum))
                w9 = gview(80)
                _boost(nc.vector.tensor_tensor(
                    out=w9.rearrange("p g (i j) -> p g i j", i=3),
                    in0=e9.rearrange("p g (i j) -> p g i j", i=3),
                    in1=esum[:, :, :, None].broadcast_to([P, 2, 3, 3]),
                    op=OP.mult,
                ))
                # weights out: w [p, i, j] -> wout[i, b, t, j]
                for g in range(2):
                    nc.sync.dma_start(
                        out=wout[:, b, t0 + g * P : t0 + (g + 1) * P, :].rearrange(
                            "i p j -> p i j"
                        ),
                        in_=sm[:, 80 + 12 * g : 80 + 12 * g + 9].rearrange(
                            "p (i j) -> p i j", i=3
                        ),
                    )

                # ---- diag matrices: diags[p, g, j, i, q] = ident[p,q]*w[p,g,3i+j]
                diags = diag_pool.tile([P, 2, 3, 3, P], F32R, tag="diags")
                for g in range(2):
                    _boost(nc.vector.tensor_tensor(
                        out=diags[:, g],
                        in0=ident[:, None, None, :].broadcast_to([P, 3, 3, P]),
                        in1=sm[:, 80 + 12 * g : 80 + 12 * g + 9]
                        .rearrange("p (i j) -> p j i", i=3)[:, :, :, None]
                        .broadcast_to([P, 3, 3, P]),
                        op=OP.mult,
                    ))

                # ---- PE combine + chunked DMA evac
                fT_sbs = []
                for g in range(2):
                    fT_sb = fT_pool.tile([P, NC4, 3, P], F32R, tag="fT")
                    for c in range(NC4):
                        # one PSUM bank per chunk so evac frees banks early
                        fT_ps = psF_pool.tile([P, 512], F32, tag="psF")
                        for j in range(3):
                            nc.tensor.matmul(
                                fT_ps[:, 0 : 3 * P],
                                lhsT=st[:, g, j, c * P : (c + 1) * P],
                                rhs=diags[:, g, j, :, :],
                                start=(j == 0),
                                stop=(j == 2),
                            )
                        # evac chunk c: [fused_0 | fused_1 | fused_2]^T
                        _boost(nc.scalar.copy(
                            out=fT_sb[:, c, :, :],
                            in_=fT_ps[:, 0 : 3 * P],
                        ))
                    fT_sbs.append(fT_sb)

                # ---- projections + residual + bn stats
                x_tiles = []
                for i in range(3):
                    x_t = x_pool.tile([P, 2, D], F32, tag="x")
                    x_tiles.append(x_t)
                    for g in range(2):
                        p_g = psP_pool.tile([P, D], F32, tag="psP")
                        for c in range(NC4):
                            nc.tensor.matmul(
                                p_g,
                                lhsT=fT_sbs[g][:, c, i, :],
                                rhs=wT_sb[:, i, c, :],
                                start=(c == 0),
                                stop=(c == NC4 - 1) and not use_pb,
                            )
                        if use_pb:
                            nc.tensor.matmul(
                                p_g,
                                lhsT=ones_row,
                                rhs=pb_sb[:, i, :],
                                start=False,
                                stop=True,
                            )
                        nc.vector.tensor_tensor(
                            out=x_t[:, g, :],
                            in0=stf[:, g, i, :],
                            in1=p_g,
                            op=OP.add,
                        )
                        k = 2 * i + g
                        nc.vector.bn_stats(
                            out=sm[:, 104 + k * 8 : 104 + k * 8 + 6],
                            in_=x_t[:, g, :],
                        )

                # ---- LN stats from bn outputs [cnt, m_e, cv_e, cnt, m_o, cv_o]
                # ws blocks of 8: 0: msum->bias, 2: d->d2h, 4: cvsum->var, 6: rstd
                bnv = sm[:, 104:152].rearrange("p (k s) -> p k s", s=8)
                wsv = sm[:, 152:200].rearrange("p (k s) -> p k s", s=8)

                def bn(s):
                    return bnv[:, :, s : s + 1]

                def ws(s):
                    return wsv[:, :, s : s + 1]

                nc.vector.tensor_tensor(out=ws(0), in0=bn(1), in1=bn(4), op=OP.add)
                nc.vector.tensor_tensor(
                    out=ws(2), in0=bn(1), in1=bn(4), op=OP.subtract
                )
                nc.vector.tensor_tensor(out=ws(4), in0=bn(2), in1=bn(5), op=OP.add)
                nc.vector.scalar_tensor_tensor(
                    out=ws(2), in0=ws(2), scalar=0.5, in1=ws(2),
                    op0=OP.mult, op1=OP.mult,
                )
                nc.vector.tensor_scalar_mul(ws(4), ws(4), 1.0 / D)
                nc.vector.scalar_tensor_tensor(
                    out=ws(4), in0=ws(2), scalar=0.5, in1=ws(4),
                    op0=OP.mult, op1=OP.add,
                )
                # rstd = exp(-0.5*ln(var + eps))
                nc.scalar.activation(out=ws(6), in_=ws(4), func=AF.Ln, bias=eps_ln)
                nc.scalar.activation(out=ws(6), in_=ws(6), func=AF.Exp, scale=-0.5)
                # bias = (msum * -0.5) * rstd
                nc.vector.scalar_tensor_tensor(
                    out=ws(0), in0=ws(0), scalar=-0.5, in1=ws(6),
                    op0=OP.mult, op1=OP.mult,
                )

                # ---- normalize + store
                for i in range(3):
                    o_t = o_pool.tile([P, 2, D], F32, tag="o")
                    for g in range(2):
                        k = 2 * i + g
                        nc.scalar.activation(
                            out=o_t[:, g, :],
                            in_=x_tiles[i][:, g, :],
                            func=AF.Identity,
                            scale=sm[:, 152 + k * 8 + 6 : 152 + k * 8 + 7],
                            bias=sm[:, 152 + k * 8 : 152 + k * 8 + 1],
                        )
                    if use_g:
                        nc.vector.tensor_mul(
                            o_t, o_t,
                            lg_sb[:, i, None, :].broadcast_to([P, 2, D]),
                        )
                    if use_b:
                        nc.vector.tensor_add(
                            o_t, o_t,
                            lb_sb[:, i, None, :].broadcast_to([P, 2, D]),
                        )
                    nc.sync.dma_start(
                        out=out[i, b, t0 : t0 + 2 * P, :].rearrange(
                            "(g p) d -> p g d", p=P
                        ),
                        in_=o_t,
                    )

    return nc


_last_info = {}


def _install_ntff_hook():
    """antenv.axon_hooks is absent in this image; recreate the registry so
    run_bass_kernel_spmd(trace=True) can reach the ctypes NTFF profiler."""
    import types

    if "antenv.axon_hooks" in sys.modules:
        return
    try:
        from trn_agent_boot.trn_boot import _ntff_profile_via_ctypes

        hook = _ntff_profile_via_ctypes("/opt/axon/libaxon_pjrt.so")
    except Exception:
        hook = None
    m = types.ModuleType("antenv.axon_hooks")
    _h = [hook]
    m.get_axon_ntff_profile_hook = lambda: _h[0]
    m.set_axon_ntff_profile_hook = lambda h: _h.__setitem__(0, h)
    sys.modules["antenv.axon_hooks"] = m


def kernel(s_flow, s_wave, s_wind, proj_w, proj_b, ln_g, ln_b):
    from concourse.bass_utils import run_bass_kernel_spmd

    B, T, D = s_flow.shape
    NCORES = 8
    B_loc = B // NCORES

    S = np.stack([np.asarray(s_flow), np.asarray(s_wave), np.asarray(s_wind)], axis=0)
    S_r = round_fp32r(S)
    WT = np.ascontiguousarray(np.transpose(np.asarray(proj_w, np.float32), (0, 2, 1)))
    WT_r = round_fp32r(WT)

    pb = np.asarray(proj_b, np.float32)
    lg = np.asarray(ln_g, np.float32)
    lb = np.asarray(ln_b, np.float32)
    use_pb = not np.all(pb == 0.0)
    use_g = not np.all(lg == 1.0)
    use_b = not np.all(lb == 0.0)

    reps = int(os.environ.get("KERNEL_REPS", "1"))
    nc = build_nc(B_loc, T, D, use_pb, use_g, use_b, reps=reps)
    legalize_waits(nc)

    in_maps = []
    for c in range(NCORES):
        m = {
            "s3": np.ascontiguousarray(S_r[:, c * B_loc : (c + 1) * B_loc]),
            "wT": WT_r,
        }
        if use_pb:
            m["pb"] = round_fp32r(pb)
        if use_g:
            m["lg"] = lg
        if use_b:
            m["lb"] = lb
        in_maps.append(m)

    trace = bool(int(os.environ.get("KERNEL_TRACE", "0")))
    if trace:
        _install_ntff_hook()
        from concourse import bass_utils as _bu

        _bu.upload_artifacts = lambda d: "local://skipped"
        tmpdir = "/root/problem/ntff_out"
        os.makedirs(tmpdir, exist_ok=True)
        res = run_bass_kernel_spmd(
            nc, in_maps, list(range(NCORES)), trace=True, tmpdir=tmpdir
        )
    else:
        res = run_bass_kernel_spmd(nc, in_maps, list(range(NCORES)))
    _last_info["exec_time_ns"] = res.exec_time_ns
    _last_info["results"] = res

    out = np.concatenate([r["out"] for r in res.results], axis=1)
    wout = np.concatenate([r["wout"] for r in res.results], axis=1)
    return out, wout
